# revision 17
# baseline (speedup 1.0000x reference)
"""MultiHeadSelfAttention2D Trainium2 kernel (8-core SPMD, full I/O).

Problem: B=4, C_IN=C_OUT=256, HEADS=8, H=W=48 (m = 2304), fp32.
  vh, zh, qh = per-head 1x1-conv projections of x; rh = fixed 2D sin/cos PE.
  scores = vh^T zh + vh^T rh  (per b,h); attn = softmax(scores/sqrt(dh), axis=n)
  out = attn @ qh^T  -> (b, c_out, h, w)

Sharding: core = 2*b + head_half. Each core handles one batch image and 4
heads (=128 output channels). No cross-core communication.

Per-core pipeline (bf16 matmul path, fp32 psum accumulation; layouts chosen
so no on-chip transposes are needed):
  - vh2/kz2 [64, 2*2304] bf16: head h at row-block 32*(h%2), col-block h//2.
    Two row blocks let two scores matmuls run concurrently on disjoint PE
    row-groups; each head owns its own psum bank (same-bank concurrent
    sub-array writes are fatal on TRN2).
  - qhT[n, ch] bf16, nb-major [128, 18*128]
  - for each 512-wide m-chunk, accumulate over 18 n-blocks in one psum pair:
      scoresT[n,m] 4 matmuls -> psc [128, 2048] (head h -> bank h)
      -> two ACT Exp ops (banks 0-1, banks 2-3) -> bf16 exp tiles; splitting
         lets next iteration's scores start while ACT drains the other half
      -> PV col-tiled 4-head matmuls + ones-matmul row-sums (M=32 replicated)
  - DVE reciprocal + multiply for the softmax normalization, DMA out
"""

import numpy as np
from contextlib import ExitStack

import concourse.bass as bass
import concourse.bacc as bacc
import concourse.tile as tile
from concourse import mybir
from concourse.bass_utils import run_bass_kernel_spmd

F32 = mybir.dt.float32
BF16 = mybir.dt.bfloat16

B, C_IN, C_OUT, HEADS, H, W = 4, 256, 256, 8, 48, 48
M = H * W  # 2304
DH = C_OUT // HEADS  # 32
HPC = 4  # heads per core
CH = HPC * DH  # 128 channels per core
NB = M // 128  # 18 n-blocks of 128
INV_SCALE = float(1.0 / np.sqrt(np.float32(DH)))  # softmax temperature

# m-chunks (free dim per matmul; last is the 2304 % 512 tail)
MCHUNKS = [(0, 512), (512, 512), (1024, 512), (1536, 512), (2048, 256)]

# blob column layout (per 128-partition row): x then the three weights
X_OFF = 0            # x  [128, 2, 2304]
WV_OFF = 2 * M       # 3x [128, 2, 128]
WZ_OFF = WV_OFF + 2 * CH
WQ_OFF = WZ_OFF + 2 * CH
BLOB_COLS = WQ_OFF + 2 * CH
# rh ships separately as [64, 2*2304] (pair layout, rows 0-63 only)


def _kernel_body(ctx: ExitStack, tc: tile.TileContext, blob_d, rh_d, out_d):
    nc = tc.nc

    consts = ctx.enter_context(tc.tile_pool(name="consts", bufs=1))
    expp = ctx.enter_context(tc.tile_pool(name="expp", bufs=6))
    outp = ctx.enter_context(tc.tile_pool(name="outp", bufs=5))
    psum_s = ctx.enter_context(tc.tile_pool(name="psum_s", bufs=1, space="PSUM"))
    psum_o = ctx.enter_context(tc.tile_pool(name="psum_o", bufs=2, space="PSUM"))

    # ---- persistent SBUF tensors ----
    blob_sb = consts.tile([128, BLOB_COLS], F32)
    rh_sb = consts.tile([64, 2 * M], F32)
    x_bf = consts.tile([128, 2, M], BF16)
    wv_bf = consts.tile([128, 2, CH], BF16)
    wz_bf = consts.tile([128, 2, CH], BF16)
    wq_bf = consts.tile([128, 2, CH], BF16)
    vh2 = consts.tile([64, 2 * M], BF16)  # [32*(h%2)+c, (h//2)*M + m]
    kz2 = consts.tile([64, 2 * M], BF16)
    qt_sb = consts.tile([128, M], BF16)   # [n, ch] nb-major blocks
    ones_sb = consts.tile([128, DH], BF16)

    nc.sync.dma_start(out=blob_sb, in_=blob_d.ap())
    nc.sync.dma_start(out=rh_sb, in_=rh_d.ap())
    nc.vector.memset(ones_sb, 1.0)

    # bf16 working copies (DVE converts; also primes DVE's clock on the DMA)
    nc.vector.tensor_copy(out=x_bf, in_=blob_sb[:, X_OFF:X_OFF + 2 * M])
    nc.vector.tensor_copy(out=wv_bf, in_=blob_sb[:, WV_OFF:WV_OFF + 2 * CH])
    nc.vector.tensor_copy(out=wz_bf, in_=blob_sb[:, WZ_OFF:WZ_OFF + 2 * CH])
    nc.vector.tensor_copy(out=wq_bf, in_=blob_sb[:, WQ_OFF:WQ_OFF + 2 * CH])

    # zero bias for Exp, produced on ACT so the exp's bias dep is a free
    # same-engine edge rather than an extra cross-engine sync wait
    zero_bias = consts.tile([128, 1], F32)
    nc.scalar.mul(out=zero_bias, in_=blob_sb[:, 0:1], mul=0.0)

    # ---- projections ----
    # vh2/kz2: per head-pair psum tile; head h -> psum rows 32*(h%2)
    for pair in range(2):
        for off, wd in MCHUNKS:
            ps = psum_s.tile([128, 2048], F32, tag="psc")
            for hh in range(2):  # head = 2*pair + hh, psum rows 32*hh
                h = 2 * pair + hh
                for k in range(2):
                    nc.tensor.matmul(
                        ps[32 * hh:32 * hh + 32, :wd],
                        lhsT=wv_bf[:, k, 32 * h:32 * h + 32],
                        rhs=x_bf[:, k, off:off + wd],
                        start=(k == 0),
                        stop=(k == 1),
                        tile_position=(0, 32 * hh),
                        skip_group_check=True,
                    )
                    nc.tensor.matmul(
                        ps[32 * hh:32 * hh + 32, 1024:1024 + wd],
                        lhsT=wz_bf[:, k, 32 * h:32 * h + 32],
                        rhs=x_bf[:, k, off:off + wd],
                        start=(k == 0),
                        stop=(k == 1),
                        tile_position=(0, 32 * hh),
                        skip_group_check=True,
                    )
            nc.vector.tensor_copy(
                out=vh2[:, pair * M + off: pair * M + off + wd], in_=ps[0:64, :wd]
            )
            nc.vector.tensor_add(
                out=kz2[:, pair * M + off: pair * M + off + wd],
                in0=ps[0:64, 1024:1024 + wd],
                in1=rh_sb[:, pair * M + off: pair * M + off + wd],
            )

    # qhT: out[n, ch] = sum_cin x[cin, n] * wT[cin, ch], per 128-wide n-block
    for nb in range(NB):
        ps = psum_s.tile([128, 2048], F32, tag="psc")
        for k in range(2):
            nc.tensor.matmul(
                ps[:, :CH],
                lhsT=x_bf[:, k, nb * 128:(nb + 1) * 128],
                rhs=wq_bf[:, k, :],
                start=(k == 0),
                stop=(k == 1),
            )
        nc.vector.tensor_copy(out=qt_sb[:, nb * 128:(nb + 1) * 128], in_=ps[:, :CH])

    # ---- attention ----
    # psc bank map (mch width w<=512): head h -> bank h, cols [512h, 512h+w)
    # scores emission: h0 (rows 0-31, bank0) || h1 (rows 32-63, bank2);
    # h2/h3 reuse the same PE rows so they serialize behind h0/h1.
    SROW = [0, 32, 0, 32]   # PE row block per head
    SBANK = [0, 2, 1, 3]    # psum bank per head

    def emit_scores(psc, j, m0, wd):
        for h in (0, 1, 2, 3):
            rb = SROW[h]
            cb = (h // 2) * M
            nc.tensor.matmul(
                psc[:, SBANK[h] * 512: SBANK[h] * 512 + wd],
                lhsT=kz2[rb:rb + 32, cb + j * 128: cb + (j + 1) * 128],
                rhs=vh2[rb:rb + 32, cb + m0: cb + m0 + wd],
                start=True,
                stop=True,
                tile_position=(rb, 0),
                skip_group_check=True,
            )

    def emit_exp(psc, wd):
        # two exp ops: banks 0-1 (heads 0, 2) then banks 2-3 (heads 1, 3);
        # each yields a [128, 2*wd] bf16 tile (head-half at cols [0,wd),[wd,2wd))
        ets = []
        for half in range(2):
            et = expp.tile([128, 1024], BF16, tag="et")
            if wd == 512:
                src = psc[:, 1024 * half: 1024 * half + 1024]
                dst = et
            else:  # tail: valid data sits at the start of each 512-col bank
                src = psc[:, 1024 * half: 1024 * half + 1024].rearrange(
                    "p (t c) -> p t c", t=2)[:, :, 0:wd]
                dst = et[:, :2 * wd].rearrange("p (t c) -> p t c", t=2)
            nc.scalar.activation(
                out=dst, in_=src,
                func=mybir.ActivationFunctionType.Exp,
                bias=zero_bias, scale=INV_SCALE,
            )
            ets.append(et)
        return ets

    def emit_pv(po, j, ets, wd):
        # ets[0] = heads 0,2 ; ets[1] = heads 1,3
        for h in range(HPC):
            et = ets[h % 2]
            ecol = (h // 2) * wd
            nc.tensor.matmul(
                po[32 * h:32 * h + 32, 0:wd],
                lhsT=qt_sb[:, j * 128 + 32 * h: j * 128 + 32 * h + 32],
                rhs=et[:, ecol:ecol + wd],
                start=(j == 0),
                stop=(j == NB - 1),
                tile_position=(0, 32 * h),
                skip_group_check=True,
            )
        for h in range(HPC):
            et = ets[h % 2]
            ecol = (h // 2) * wd
            nc.tensor.matmul(
                po[32 * h:32 * h + 32, 512:512 + wd],
                lhsT=ones_sb,
                rhs=et[:, ecol:ecol + wd],
                start=(j == 0),
                stop=(j == NB - 1),
                tile_position=(0, 32 * h),
                skip_group_check=True,
            )

    for m0, wd in MCHUNKS:
        po = psum_o.tile([128, 1024], F32, tag="po")
        prev = None
        for j in range(NB):
            psc = psum_s.tile([128, 2048], F32, tag="psc")
            emit_scores(psc, j, m0, wd)
            ets = emit_exp(psc, wd)
            if prev is not None:
                emit_pv(po, prev[0], prev[1], wd)
            prev = (j, ets)
        emit_pv(po, prev[0], prev[1], wd)

        rc = outp.tile([128, 512], F32, tag="recip")
        nc.vector.reciprocal(out=rc[:, :wd], in_=po[:, 512:512 + wd])
        of = outp.tile([128, 512], F32, tag="outf")
        nc.vector.tensor_mul(out=of[:, :wd], in0=po[:, 0:wd], in1=rc[:, :wd])
        nc.sync.dma_start(out=out_d.ap()[:, m0:m0 + wd], in_=of[:, :wd])


def build_module() -> bass.Bass:
    nc = bacc.Bacc("TRN2", target_bir_lowering=False)
    blob_d = nc.declare_dram_parameter("blob", [128, BLOB_COLS], F32, isOutput=False)
    rh_d = nc.declare_dram_parameter("rh2", [64, 2 * M], F32, isOutput=False)
    out_d = nc.declare_dram_parameter("out", [CH, M], F32, isOutput=True)
    with tile.TileContext(nc) as tc, ExitStack() as ctx:
        _kernel_body(ctx, tc, blob_d, rh_d, out_d)
    nc.compile()
    return nc


def pos_encoding_2d(c, h, w):
    """numpy port of the reference's fixed 2D sinusoidal PE -> (c, h*w)."""
    ch = c // 2
    div = np.float32(10000.0) ** (np.arange(0, ch, 2, dtype=np.float32) / np.float32(ch))
    py = np.arange(h, dtype=np.float32)[None, :] / div[:, None]
    px = np.arange(w, dtype=np.float32)[None, :] / div[:, None]
    pe_y = np.stack([np.sin(py), np.cos(py)], axis=1).reshape(ch, h).astype(np.float32)
    pe_x = np.stack([np.sin(px), np.cos(px)], axis=1).reshape(ch, w).astype(np.float32)
    pe = np.concatenate(
        [
            np.broadcast_to(pe_y[:, :, None], (ch, h, w)),
            np.broadcast_to(pe_x[:, None, :], (ch, h, w)),
        ],
        axis=0,
    )
    return np.ascontiguousarray(pe.reshape(c, h * w), dtype=np.float32)


_CACHE = {}


def _get_nc() -> bass.Bass:
    if "nc" not in _CACHE:
        _CACHE["nc"] = build_module()
    return _CACHE["nc"]


def make_in_maps(x, w_v, w_z, w_q):
    rh_full = pos_encoding_2d(C_OUT, H, W)
    x = np.asarray(x, dtype=np.float32)
    w_v = np.asarray(w_v, dtype=np.float32)
    w_z = np.asarray(w_z, dtype=np.float32)
    w_q = np.asarray(w_q, dtype=np.float32)
    in_maps = []
    for core in range(8):
        b, hh = core // 2, core % 2
        c0 = CH * hh
        blob = np.zeros((128, BLOB_COLS), np.float32)
        # x: blob[p, k*M + j] = x[b, k*128 + p, j]
        xx = x[b].reshape(2, 128, M)
        blob[:, X_OFF:X_OFF + 2 * M] = xx.transpose(1, 0, 2).reshape(128, 2 * M)
        # weights: blob[p, base + k*CH + c] = w[c0 + c, k*128 + p]
        for base, wm in ((WV_OFF, w_v), (WZ_OFF, w_z), (WQ_OFF, w_q)):
            wt = wm[c0:c0 + CH, :].T.reshape(2, 128, CH)  # [k, p, c]
            blob[:, base:base + 2 * CH] = wt.transpose(1, 0, 2).reshape(128, 2 * CH)
        # rh2: [32*(h%2)+c, (h//2)*M + m] = rh[c0 + 32h + c, m]  (rows 0-63)
        rh2 = np.zeros((64, 2 * M), np.float32)
        rh_c = rh_full[c0:c0 + CH, :].reshape(4, DH, M)  # [h, c, m]
        for h in range(4):
            r0 = DH * (h % 2)
            cb = (h // 2) * M
            rh2[r0:r0 + DH, cb:cb + M] = rh_c[h]
        in_maps.append({"blob": blob, "rh2": rh2})
    return in_maps


def assemble_output(results):
    out = np.empty((B, C_OUT, H, W), np.float32)
    for core in range(8):
        b, hh = core // 2, core % 2
        out[b, CH * hh:CH * hh + CH] = results[core]["out"].reshape(CH, H, W)
    return out


def kernel(x, w_v, w_z, w_q, _trace=False):
    nc = _get_nc()
    in_maps = make_in_maps(x, w_v, w_z, w_q)
    res = run_bass_kernel_spmd(nc, in_maps, core_ids=list(range(8)), trace=_trace)
    out = assemble_output(res.results)
    if _trace:
        kernel.last_results = res
    return out


# revision 19
# speedup vs baseline: 1.5468x; 1.5468x over previous
"""MultiHeadSelfAttention2D Trainium2 kernel (8-core SPMD, full I/O).

Problem: B=4, C_IN=C_OUT=256, HEADS=8, H=W=48 (m = 2304), fp32.
  vh, zh, qh = per-head 1x1-conv projections of x; rh = fixed 2D sin/cos PE.
  scores = vh^T zh + vh^T rh  (per b,h); attn = softmax(scores/sqrt(dh), axis=n)
  out = attn @ qh^T  -> (b, c_out, h, w)

Sharding: core = 2*b + head_half. Each core handles one batch image and 4
heads (=128 output channels). No cross-core communication.

Per-core pipeline (bf16 matmul path, fp32 psum accumulation; layouts chosen
so no on-chip transposes are needed):
  - vh2/kz2 [64, 2*2304] bf16: head h at row-block 32*(h%2), col-block h//2.
    Two row blocks let two scores matmuls run concurrently on disjoint PE
    row-groups while writing different psum banks (same-bank concurrent
    sub-array writes are fatal on TRN2).
  - qhT[n, ch] bf16, nb-major [128, 18*128]
  - for each 256-wide m-chunk, accumulate over 18 n-blocks in one psum bank:
      scoresT[n,m] 4 matmuls (2-way row-concurrent) -> psc [128, 1024]
      -> one ACT Exp (psum->sbuf bf16; ACT is the bottleneck engine)
      -> PV col-tiled 4-head matmuls; row-sums via ones-matmul every OTHER
         n-block on a DVE-precomputed et(j)+et(j+1) pair (halves PE's
         sums matmuls; PE runs HAM-throttled at 1.2 GHz on this workload)
  - DVE reciprocal + multiply for the softmax normalization, DMA out
"""

import numpy as np
from contextlib import ExitStack

import concourse.bass as bass
import concourse.bacc as bacc
import concourse.tile as tile
from concourse import mybir
from concourse.bass_utils import run_bass_kernel_spmd

F32 = mybir.dt.float32
BF16 = mybir.dt.bfloat16

B, C_IN, C_OUT, HEADS, H, W = 4, 256, 256, 8, 48, 48
M = H * W  # 2304
DH = C_OUT // HEADS  # 32
HPC = 4  # heads per core
CH = HPC * DH  # 128 channels per core
NB = M // 128  # 18 n-blocks of 128
MCH = 256  # m-chunk width
NMCH = M // MCH  # 9
INV_SCALE = float(1.0 / np.sqrt(np.float32(DH)))  # softmax temperature

PROJ_CHUNKS = [(0, 512), (512, 512), (1024, 512), (1536, 512), (2048, 256)]

# blob column layout (per 128-partition row): x then the three weights
X_OFF = 0            # x  [128, 2, 2304]
WV_OFF = 2 * M       # 3x [128, 2, 128]
WZ_OFF = WV_OFF + 2 * CH
WQ_OFF = WZ_OFF + 2 * CH
BLOB_COLS = WQ_OFF + 2 * CH
# rh ships separately as [64, 2*2304] (pair layout, rows 0-63 only)

# scores psum / exp-tile column block per head: two-way row concurrency,
# h0/h2 share psum bank 0 (row block 0), h1/h3 share bank 1 (row block 32)
ECOL = [0, 2, 1, 3]  # head -> 256-col block in psc/et


def _kernel_body(ctx: ExitStack, tc: tile.TileContext, blob_d, rh_d, out_d):
    nc = tc.nc

    consts = ctx.enter_context(tc.tile_pool(name="consts", bufs=1))
    expp = ctx.enter_context(tc.tile_pool(name="expp", bufs=6))
    sump = ctx.enter_context(tc.tile_pool(name="sump", bufs=3))
    outp = ctx.enter_context(tc.tile_pool(name="outp", bufs=9))
    psum_s = ctx.enter_context(tc.tile_pool(name="psum_s", bufs=3, space="PSUM"))
    psum_o = ctx.enter_context(tc.tile_pool(name="psum_o", bufs=2, space="PSUM"))

    # ---- persistent SBUF tensors ----
    blob_sb = consts.tile([128, BLOB_COLS], F32)
    rh_sb = consts.tile([64, 2 * M], F32)
    x_bf = consts.tile([128, 2, M], BF16)
    wv_bf = consts.tile([128, 2, CH], BF16)
    wz_bf = consts.tile([128, 2, CH], BF16)
    wq_bf = consts.tile([128, 2, CH], BF16)
    vh2 = consts.tile([64, 2 * M], BF16)  # [32*(h%2)+c, (h//2)*M + m]
    kz2 = consts.tile([64, 2 * M], BF16)
    qt_sb = consts.tile([128, M], BF16)   # [n, ch] nb-major blocks
    ones_sb = consts.tile([128, DH], BF16)

    # weights + rh land first (small), then x in chunks so the projections
    # can start while the rest of x is still in flight
    nc.sync.dma_start(
        out=blob_sb[:, WV_OFF:BLOB_COLS],
        in_=blob_d.ap()[:, WV_OFF:BLOB_COLS],
    )
    nc.sync.dma_start(out=rh_sb, in_=rh_d.ap())
    for k in range(2):
        for off, wd in PROJ_CHUNKS:
            nc.sync.dma_start(
                out=blob_sb[:, k * M + off: k * M + off + wd],
                in_=blob_d.ap()[:, k * M + off: k * M + off + wd],
            )
    nc.vector.memset(ones_sb, 1.0)

    # bf16 working copies (DVE converts)
    nc.vector.tensor_copy(out=wv_bf, in_=blob_sb[:, WV_OFF:WV_OFF + 2 * CH])
    nc.vector.tensor_copy(out=wz_bf, in_=blob_sb[:, WZ_OFF:WZ_OFF + 2 * CH])
    nc.vector.tensor_copy(out=wq_bf, in_=blob_sb[:, WQ_OFF:WQ_OFF + 2 * CH])
    for k in range(2):
        for off, wd in PROJ_CHUNKS:
            nc.vector.tensor_copy(
                out=x_bf[:, k, off:off + wd],
                in_=blob_sb[:, k * M + off: k * M + off + wd],
            )

    # zero bias for Exp, produced on ACT so the exp's bias dep is a free
    # same-engine edge rather than an extra cross-engine sync wait
    zero_bias = consts.tile([128, 1], F32)
    nc.scalar.mul(out=zero_bias, in_=blob_sb[:, WV_OFF:WV_OFF + 1], mul=0.0)

    # ---- projections ----
    # vh2/kz2: head h -> psum rows 32*(h%2); vh in bank 0, kz in bank 2
    for pair in range(2):
        for off, wd in PROJ_CHUNKS:
            ps = psum_s.tile([128, 4 * MCH], F32, tag="psc")
            for hh in range(2):  # head = 2*pair + hh
                h = 2 * pair + hh
                for k in range(2):
                    nc.tensor.matmul(
                        ps[32 * hh:32 * hh + 32, :wd],
                        lhsT=wv_bf[:, k, 32 * h:32 * h + 32],
                        rhs=x_bf[:, k, off:off + wd],
                        start=(k == 0),
                        stop=(k == 1),
                        tile_position=(0, 32 * hh),
                        skip_group_check=True,
                    )
                    nc.tensor.matmul(
                        ps[32 * hh:32 * hh + 32, 512:512 + wd],
                        lhsT=wz_bf[:, k, 32 * h:32 * h + 32],
                        rhs=x_bf[:, k, off:off + wd],
                        start=(k == 0),
                        stop=(k == 1),
                        tile_position=(0, 32 * hh),
                        skip_group_check=True,
                    )
            nc.vector.tensor_copy(
                out=vh2[:, pair * M + off: pair * M + off + wd], in_=ps[0:64, :wd]
            )
            nc.vector.tensor_add(
                out=kz2[:, pair * M + off: pair * M + off + wd],
                in0=ps[0:64, 512:512 + wd],
                in1=rh_sb[:, pair * M + off: pair * M + off + wd],
            )

    # qhT: out[n, ch] = sum_cin x[cin, n] * wT[cin, ch], per 128-wide n-block
    for nb in range(NB):
        ps = psum_s.tile([128, 4 * MCH], F32, tag="psc")
        for k in range(2):
            nc.tensor.matmul(
                ps[:, :CH],
                lhsT=x_bf[:, k, nb * 128:(nb + 1) * 128],
                rhs=wq_bf[:, k, :],
                start=(k == 0),
                stop=(k == 1),
            )
        nc.vector.tensor_copy(out=qt_sb[:, nb * 128:(nb + 1) * 128], in_=ps[:, :CH])

    # ---- attention ----
    def emit_pv(po, j, et):
        # PV: out_unnorm[32h+d, m] += sum_n qhT[n, 32h+d] * expT_h[n, m]
        for h in range(HPC):
            nc.tensor.matmul(
                po[32 * h:32 * h + 32, 0:MCH],
                lhsT=qt_sb[:, j * 128 + 32 * h: j * 128 + 32 * h + 32],
                rhs=et[:, ECOL[h] * MCH:(ECOL[h] + 1) * MCH],
                start=(j == 0),
                stop=False,
                tile_position=(0, 32 * h),
                skip_group_check=True,
            )

    def emit_sums(po, etp, last):
        # row-sums of an et(j)+et(j+1) pair, replicated over each head's slot
        for h in range(HPC):
            nc.tensor.matmul(
                po[32 * h:32 * h + 32, MCH:2 * MCH],
                lhsT=ones_sb,
                rhs=etp[:, ECOL[h] * MCH:(ECOL[h] + 1) * MCH],
                start=False,
                stop=last,
                tile_position=(0, 32 * h),
                skip_group_check=True,
            )

    def emit_pair_sums(po, ets, pj, last):
        etp = sump.tile([128, 4 * MCH], BF16, tag="etp")
        nc.vector.tensor_add(out=etp, in0=ets[pj - 1], in1=ets[pj])
        emit_sums(po, etp, last)

    for mc in range(NMCH):
        m0 = mc * MCH
        po = psum_o.tile([128, 2 * MCH], F32, tag="po")
        ets = {}
        for j in range(NB):
            psc = psum_s.tile([128, 4 * MCH], F32, tag="psc")
            # h0 (rows 0-31 -> bank0) || h1 (rows 32-63 -> bank1) concurrent;
            # h2/h3 reuse the same PE rows so they serialize behind h0/h1.
            for h in (0, 1, 2, 3):
                rb = 32 * (h % 2)
                cb = (h // 2) * M
                nc.tensor.matmul(
                    psc[:, ECOL[h] * MCH:(ECOL[h] + 1) * MCH],
                    lhsT=kz2[rb:rb + 32, cb + j * 128: cb + (j + 1) * 128],
                    rhs=vh2[rb:rb + 32, cb + m0: cb + m0 + MCH],
                    start=True,
                    stop=True,
                    tile_position=(rb, 0),
                    skip_group_check=True,
                )
            et = expp.tile([128, 4 * MCH], BF16, tag="et")
            nc.scalar.activation(
                out=et, in_=psc, func=mybir.ActivationFunctionType.Exp,
                bias=zero_bias, scale=INV_SCALE,
            )
            ets[j] = et
            # lag PV/sums one j behind so PE never head-of-line blocks on ACT
            if j >= 1:
                emit_pv(po, j - 1, ets[j - 1])
                if (j - 1) % 2 == 1:
                    emit_pair_sums(po, ets, j - 1, last=False)
        emit_pv(po, NB - 1, ets[NB - 1])
        emit_pair_sums(po, ets, NB - 1, last=True)

        rc = outp.tile([128, MCH], F32, tag="recip")
        nc.vector.reciprocal(out=rc, in_=po[:, MCH:2 * MCH])
        of = outp.tile([128, MCH], F32, tag="outf")
        nc.vector.tensor_mul(out=of, in0=po[:, 0:MCH], in1=rc)
        nc.sync.dma_start(out=out_d.ap()[:, m0:m0 + MCH], in_=of)


def build_module() -> bass.Bass:
    nc = bacc.Bacc("TRN2", target_bir_lowering=False)
    blob_d = nc.declare_dram_parameter("blob", [128, BLOB_COLS], F32, isOutput=False)
    rh_d = nc.declare_dram_parameter("rh2", [64, 2 * M], F32, isOutput=False)
    out_d = nc.declare_dram_parameter("out", [CH, M], F32, isOutput=True)
    with tile.TileContext(nc) as tc, ExitStack() as ctx:
        _kernel_body(ctx, tc, blob_d, rh_d, out_d)
    nc.compile()
    return nc


def pos_encoding_2d(c, h, w):
    """numpy port of the reference's fixed 2D sinusoidal PE -> (c, h*w)."""
    ch = c // 2
    div = np.float32(10000.0) ** (np.arange(0, ch, 2, dtype=np.float32) / np.float32(ch))
    py = np.arange(h, dtype=np.float32)[None, :] / div[:, None]
    px = np.arange(w, dtype=np.float32)[None, :] / div[:, None]
    pe_y = np.stack([np.sin(py), np.cos(py)], axis=1).reshape(ch, h).astype(np.float32)
    pe_x = np.stack([np.sin(px), np.cos(px)], axis=1).reshape(ch, w).astype(np.float32)
    pe = np.concatenate(
        [
            np.broadcast_to(pe_y[:, :, None], (ch, h, w)),
            np.broadcast_to(pe_x[:, None, :], (ch, h, w)),
        ],
        axis=0,
    )
    return np.ascontiguousarray(pe.reshape(c, h * w), dtype=np.float32)


_CACHE = {}


def _get_nc() -> bass.Bass:
    if "nc" not in _CACHE:
        _CACHE["nc"] = build_module()
    return _CACHE["nc"]


def make_in_maps(x, w_v, w_z, w_q):
    rh_full = pos_encoding_2d(C_OUT, H, W)
    x = np.asarray(x, dtype=np.float32)
    w_v = np.asarray(w_v, dtype=np.float32)
    w_z = np.asarray(w_z, dtype=np.float32)
    w_q = np.asarray(w_q, dtype=np.float32)
    in_maps = []
    for core in range(8):
        b, hh = core // 2, core % 2
        c0 = CH * hh
        blob = np.zeros((128, BLOB_COLS), np.float32)
        # x: blob[p, k*M + j] = x[b, k*128 + p, j]
        xx = x[b].reshape(2, 128, M)
        blob[:, X_OFF:X_OFF + 2 * M] = xx.transpose(1, 0, 2).reshape(128, 2 * M)
        # weights: blob[p, base + k*CH + c] = w[c0 + c, k*128 + p]
        for base, wm in ((WV_OFF, w_v), (WZ_OFF, w_z), (WQ_OFF, w_q)):
            wt = wm[c0:c0 + CH, :].T.reshape(2, 128, CH)  # [k, p, c]
            blob[:, base:base + 2 * CH] = wt.transpose(1, 0, 2).reshape(128, 2 * CH)
        # rh2: [32*(h%2)+c, (h//2)*M + m] = rh[c0 + 32h + c, m]  (rows 0-63)
        rh2 = np.zeros((64, 2 * M), np.float32)
        rh_c = rh_full[c0:c0 + CH, :].reshape(4, DH, M)  # [h, c, m]
        for h in range(4):
            r0 = DH * (h % 2)
            cb = (h // 2) * M
            rh2[r0:r0 + DH, cb:cb + M] = rh_c[h]
        in_maps.append({"blob": blob, "rh2": rh2})
    return in_maps


def assemble_output(results):
    out = np.empty((B, C_OUT, H, W), np.float32)
    for core in range(8):
        b, hh = core // 2, core % 2
        out[b, CH * hh:CH * hh + CH] = results[core]["out"].reshape(CH, H, W)
    return out


def kernel(x, w_v, w_z, w_q, _trace=False):
    nc = _get_nc()
    in_maps = make_in_maps(x, w_v, w_z, w_q)
    res = run_bass_kernel_spmd(nc, in_maps, core_ids=list(range(8)), trace=_trace)
    out = assemble_output(res.results)
    if _trace:
        kernel.last_results = res
    return out
